# revision 10
# baseline (speedup 1.0000x reference)
"""Trainium2 Bass kernel for nn_Act3dEncoder (dense_transformer).

Sharding: data-parallel over batch B=32 across 8 NeuronCores (4 batches/core),
params replicated. All compute on device. Key algorithmic restructurings
(validated against the reference to ~7e-7 in fp32):

 - Activations kept feature-transposed ([feat, n] on partitions) so every GEMM
   contracts on partitions with the weights stationary.
 - Feature dims de-interleaved (evens|odds) via host-side row permutation of
   Wq/Wk so RoPE pair structure becomes two aligned [120, n] chunks.
 - Rotation folded into the PSUM evacuation: CK/SK = (K + bk) * cos/sin via
   one scalar_tensor_tensor pass each; scores computed as a 4-chunk augmented
   contraction against head-blocked rotated queries (zeros off-head), which
   also realizes the per-head block-diagonal attention in one matmul.
 - attn@V collapsed via attnF: af = softmax-unnormalized(exp(s)) @ feat, then
   av = (af/Z) @ Wv^T + bv (softmax rows sum to 1). Eliminates all V
   projections over N entirely.
 - Scores are small (|s| < 0.5 for this model), so exp() without
   max-subtraction; Z accumulated for free via the ACT accum_out port.
 - fp32r (full-rate) for the big matmuls, bf16 for cos/sin tables, featrow and
   exp(scores); everything else fp32.
"""

import sys

sys.path.insert(0, "/opt/trn_rl_repo")

import numpy as np
import ml_dtypes

B, N, D, H, NGP = 32, 8192, 240, 3, 4
HD = D // H          # 80
NC_COUNT = 8
BL = B // NC_COUNT   # 4 batches per core
PAIRS = D // 2       # 120
T = 512              # n-tile
NT = N // T          # 16
OUTF = NGP * D + 64 + NGP * D  # 1984
F32 = np.float32

_PERM = np.concatenate([np.arange(0, D, 2), np.arange(1, D, 2)])  # evens|odds


def _prep_consts(params):
    """Host-side weight layout prep. Returns dict name -> np.ndarray."""
    c = {}
    vis = params["vision"]
    # vision layer 1: W [240, 3] -> lhsT [3, 240]
    c["w1t"] = np.ascontiguousarray(vis[0]["W"].T, F32)  # [3, 240]
    for i, nm in ((1, "w2t"), (2, "w3t")):
        wt = vis[i]["W"].T.astype(F32)  # [240 in, 240 out]
        c[nm] = np.ascontiguousarray(
            wt.reshape(2, 120, 240).transpose(1, 0, 2))  # [cp, cc, out]
    vb = np.stack([vis[i]["b"].astype(F32) for i in range(3)], -1)  # [240, 3]
    c["vb"] = np.ascontiguousarray(vb.reshape(2, 120, 3).transpose(1, 0, 2))  # [120,2,3]

    # RoPE frequency selection matrix: ang[p, n] = sum_a msel[a, p] * xyz[a, n]
    d3 = D // 3
    div = np.exp(np.arange(0, d3, 2, dtype=F32) * (-np.log(10000.0) / d3))  # [40]
    msel = np.zeros((3, PAIRS), F32)
    for p in range(PAIRS):
        msel[p // 40, p] = div[p % 40]
    c["msel"] = msel
    c["msel_r"] = msel.copy()

    # attention weights: lm = module*2 + layer
    wq = np.zeros((120, 4, 2, 240), F32)
    wk = np.zeros((120, 4, 2, 240), F32)
    wv = np.zeros((120, 4, 2, 240), F32)
    wo = np.zeros((120, 4, 2, 240), F32)
    bq = np.zeros((120, 4, 2), F32)
    bk = np.zeros((120, 4, 2), F32)
    bv = np.zeros((120, 4, 2), F32)
    gbb = np.zeros((4, 3, 240), F32)  # [lm, (bo,g,beta), d]
    scale = HD ** -0.5
    for m, key in ((0, "attn"), (1, "goal_attn")):
        for l, p in enumerate(params[key]):
            lm = m * 2 + l
            Wq, Wk, Wv = np.split(np.asarray(p["Wqkv"], F32), 3, axis=0)
            bq_, bk_, bv_ = np.split(np.asarray(p["bqkv"], F32), 3)
            wqt = (Wq * scale).T[:, _PERM]   # [240c, 240 out-permuted]
            wkt = Wk.T[:, _PERM]
            wvt = Wv.T                        # natural out
            wot = np.asarray(p["Wo"], F32).T  # [240 in(avsel d), 240 out]
            for arr, w in ((wq, wqt), (wk, wkt), (wv, wvt), (wo, wot)):
                arr[:, lm] = w.reshape(2, 120, 240).transpose(1, 0, 2)
            bq[:, lm] = (bq_ * scale)[_PERM].reshape(2, 120).T
            bk[:, lm] = bk_[_PERM].reshape(2, 120).T
            bv[:, lm] = bv_.reshape(2, 120).T
            gbb[lm, 0] = p["bo"]
            gbb[lm, 1] = p["g"]
            gbb[lm, 2] = p["beta"]
    c["wq"], c["wv"], c["wo"] = wq, wv, wo
    c["wk"] = wk.astype(ml_dtypes.bfloat16)
    c["bq"], c["bk"], c["bv"], c["gbb"] = bq, bk, bv, gbb

    # gripper / goal PE MLPs [6 -> 128 -> 256 -> 80]
    gw1 = np.zeros((6, 2, 128), F32)
    gb1 = np.zeros((128, 2), F32)
    gw2 = np.zeros((128, 2, 256), F32)
    gb2 = np.zeros((128, 4), F32)   # [p, which*2 + oc]
    gw3 = np.zeros((128, 2, 2, 80), F32)
    gb3 = np.zeros((80, 2), F32)
    for w, key in ((0, "gripper_pe_mlp"), (1, "goal_pe_mlp")):
        ps = params[key]
        gw1[:, w] = np.asarray(ps[0]["W"], F32).T
        gb1[:, w] = ps[0]["b"]
        gw2[:, w] = np.asarray(ps[1]["W"], F32).T
        gb2[:, w * 2 + 0] = ps[1]["b"][:128]
        gb2[:, w * 2 + 1] = ps[1]["b"][128:]
        w3t = np.asarray(ps[2]["W"], F32).T  # [256, 80]
        gw3[:, w, 0] = w3t[:128]
        gw3[:, w, 1] = w3t[128:]
        gb3[:, w] = ps[2]["b"]
    c["gw1"], c["gb1"], c["gw2"], c["gb2"] = gw1, gb1, gw2, gb2
    c["gw3"], c["gb3"] = gw3, gb3
    c["embeds"] = np.stack([np.asarray(params["embed"], F32),
                            np.asarray(params["goal_embed"], F32)])  # [2, 160]

    st = params["state_mlp"]
    c["sw1"] = np.ascontiguousarray(np.asarray(st[0]["W"], F32).T)  # [8, 64]
    c["sw2"] = np.ascontiguousarray(np.asarray(st[1]["W"], F32).T)  # [64, 64]
    c["sb"] = np.stack([np.asarray(st[0]["b"], F32),
                        np.asarray(st[1]["b"], F32)], -1)  # [64, 2]
    return c


def _build_program(consts):
    import ml_dtypes
    import concourse.bass as bass
    import concourse.bacc as bacc
    import concourse.mybir as mybir
    import concourse.tile as tile
    from concourse.masks import make_identity
    from contextlib import ExitStack

    f32 = mybir.dt.float32
    f32r = mybir.dt.float32r
    bf16 = mybir.dt.bfloat16
    AF = mybir.ActivationFunctionType
    OP = mybir.AluOpType
    AX = mybir.AxisListType

    nc = bacc.Bacc(None, target_bir_lowering=False)

    pc_h = nc.declare_dram_parameter("pc", [BL, N, 3], f32, isOutput=False)
    grip_h = nc.declare_dram_parameter("grip", [BL, NGP, 3], f32, isOutput=False)
    goal_h = nc.declare_dram_parameter("goal", [BL, NGP, 3], f32, isOutput=False)
    agent_h = nc.declare_dram_parameter("agent", [BL, 8], f32, isOutput=False)
    F32R_W = {"w1t", "w2t", "w3t", "msel_r"}
    w_h = {}
    for name, arr in consts.items():
        if arr.dtype == ml_dtypes.bfloat16:
            dt_ = bf16
        elif name in F32R_W:
            dt_ = f32r
        else:
            dt_ = f32
        w_h[name] = nc.declare_dram_parameter(name, list(arr.shape), dt_,
                                              isOutput=False)
    out_h = nc.declare_dram_parameter("out", [BL, OUTF], f32, isOutput=True)

    def bcast(handle, p):
        ap = handle[:]
        return bass.AP(tensor=ap.tensor, offset=ap.offset,
                       ap=[[0, p]] + list(ap.ap))

    def r(ap):
        return ap.bitcast(f32r)

    with tile.TileContext(nc) as tc, ExitStack() as ctx:
        wp = ctx.enter_context(tc.tile_pool(name="wp", bufs=1))
        persist = ctx.enter_context(tc.tile_pool(name="persist", bufs=1))
        sb = ctx.enter_context(tc.tile_pool(name="sb", bufs=2))
        ck = ctx.enter_context(tc.tile_pool(name="ck", bufs=2))
        qs = ctx.enter_context(tc.tile_pool(name="qs", bufs=2))
        psum = ctx.enter_context(tc.tile_pool(name="psum", bufs=2, space="PSUM"))

        # ---- load weights to SBUF ----
        W = {}
        for name, arr in consts.items():
            if arr.dtype == ml_dtypes.bfloat16:
                dt_ = bf16
            elif name in F32R_W:
                dt_ = f32r
            else:
                dt_ = f32
            tile_ = wp.tile(list(arr.shape), dt_, tag=f"w_{name}", name=f"w_{name}")
            nc.sync.dma_start(out=tile_, in_=w_h[name][:])
            W[name] = tile_
        ident = wp.tile([128, 128], f32, tag="ident")
        make_identity(nc, ident)
        ident_bf = wp.tile([128, 128], bf16, tag="ident_bf")
        make_identity(nc, ident_bf)
        pihalf = wp.tile([128, 1], f32, tag="pihalf")
        nc.vector.memset(pihalf, float(np.pi / 2))
        epst = wp.tile([NGP, 1], f32, tag="epst")
        nc.vector.memset(epst, 1e-5)
        zblk = wp.tile([120, 12], f32, tag="zblk")
        nc.vector.memset(zblk, 0.0)
        # broadcast constants over rows
        emb_s = wp.tile([NGP, 2, 160], f32, tag="emb_s")
        nc.gpsimd.dma_start(out=emb_s, in_=bcast(w_h["embeds"], NGP))

        # ---- state MLP (all 4 batches at once) ----
        sT = qs.tile([8, BL], f32, tag="sT")
        with nc.allow_non_contiguous_dma(reason="tiny transposed load"):
            nc.sync.dma_start(out=sT, in_=agent_h[:].rearrange("b f -> f b"))
        s1p = psum.tile([64, BL], f32, tag="ps")
        nc.tensor.matmul(s1p, lhsT=W["sw1"][:], rhs=sT, start=True, stop=True)
        s1 = qs.tile([64, BL], f32, tag="s1")
        nc.scalar.activation(s1, s1p, AF.Relu, bias=W["sb"][:, 0:1])
        s2p = psum.tile([64, BL], f32, tag="ps")
        nc.tensor.matmul(s2p, lhsT=W["sw2"][:], rhs=s1, start=True, stop=True)
        s2 = qs.tile([64, BL], f32, tag="s2")
        nc.vector.tensor_scalar_add(s2, s2p, W["sb"][:, 1:2])
        strp = psum.tile([BL, 64], f32, tag="ps")
        nc.tensor.transpose(strp, s2, ident[:64, :64])
        strow = qs.tile([BL, 64], f32, tag="strow")
        nc.vector.tensor_copy(strow, strp)
        nc.sync.dma_start(out=out_h[:, NGP * D:NGP * D + 64], in_=strow)

        for b in range(BL):
            # ======== vision + rope tables for batch b ========
            featT = [persist.tile([120, N], bf16, tag=f"featT{cc}", name=f"featT{cc}")
                     for cc in range(2)]
            cosP = persist.tile([PAIRS, N], bf16, tag="cosP")
            sinP = persist.tile([PAIRS, N], bf16, tag="sinP")
            frow = persist.tile([128, N // 128, 240], bf16, tag="frow")

            pct = persist.tile([128, N // 128, 3], f32, tag="pct")
            nc.sync.dma_start(
                out=pct, in_=pc_h[b].rearrange("(cb p) c -> p cb c", p=128))

            for t in range(NT):
                n0 = t * T
                xyzT = sb.tile([3, T], f32r, tag="xyzT")
                for j in range(4):
                    cb = t * 4 + j
                    tp = psum.tile([3, 128], f32, tag="vis")
                    nc.tensor.transpose(tp, pct[:, cb, :], ident)
                    nc.vector.tensor_copy(xyzT[:, j * 128:(j + 1) * 128], tp)
                # rope angles -> cos/sin
                angp = psum.tile([PAIRS, T], f32, tag="ps")
                nc.tensor.matmul(angp, lhsT=W["msel_r"][:], rhs=xyzT,
                                 start=True, stop=True)
                nc.scalar.activation(sinP[:, n0:n0 + T], angp, AF.Sin)
                nc.scalar.activation(cosP[:, n0:n0 + T], angp, AF.Sin,
                                     bias=pihalf[:PAIRS])
                # vision MLP
                x1 = sb.tile([120, 2, T], f32r, tag="x1")
                for oc in range(2):
                    p1 = psum.tile([120, T], f32, tag="K")
                    nc.tensor.matmul(p1, lhsT=W["w1t"][:, oc * 120:(oc + 1) * 120],
                                     rhs=xyzT, start=True, stop=True)
                    # bias+relu on DVE: (x + b) max 0
                    nc.vector.tensor_scalar(
                        out=x1[:, oc], in0=p1, scalar1=W["vb"][:, oc, 0:1],
                        scalar2=0.0, op0=OP.add, op1=OP.max)
                x2 = sb.tile([120, 2, T], f32r, tag="x2")
                for oc in range(2):
                    p2 = psum.tile([120, T], f32, tag="K")
                    for cc in range(2):
                        nc.tensor.matmul(
                            p2, lhsT=W["w2t"][:, cc, oc * 120:(oc + 1) * 120],
                            rhs=x1[:, cc], start=(cc == 0), stop=(cc == 1))
                    nc.scalar.activation(x2[:, oc], p2, AF.Relu,
                                         bias=W["vb"][:, oc, 1:2])
                for oc in range(2):
                    p3 = psum.tile([120, T], f32, tag="K")
                    for cc in range(2):
                        nc.tensor.matmul(
                            p3, lhsT=W["w3t"][:, cc, oc * 120:(oc + 1) * 120],
                            rhs=x2[:, cc], start=(cc == 0), stop=(cc == 1))
                    nc.vector.tensor_scalar_add(featT[oc][:, n0:n0 + T], p3,
                                                W["vb"][:, oc, 2:3])
                # featrow: transpose featT tile chunks into [n, 240] bf16
                for j in range(4):
                    s0 = n0 + j * 128
                    for oc in range(2):
                        fp = psum.tile([128, 120], bf16, tag="af")
                        nc.tensor.transpose(fp, featT[oc][:, s0:s0 + 128],
                                            ident_bf[:120, :120])
                        nc.scalar.copy(frow[:, t * 4 + j, oc * 120:(oc + 1) * 120], fp)

            # ======== gripper-side module inputs ========
            in6 = {}
            cosq, sinq = {}, {}
            xstate = {}
            for m in range(2):
                in6[m] = qs.tile([6, NGP], f32, tag=f"in6_{m}", name=f"in6_{m}")
            gxyz = qs.tile([3, NGP], f32, tag="gxyz")
            oxyz = qs.tile([3, NGP], f32, tag="oxyz")
            dispt = qs.tile([3, NGP], f32, tag="dispt")
            with nc.allow_non_contiguous_dma(reason="tiny transposed load"):
                nc.sync.dma_start(out=gxyz, in_=grip_h[b].rearrange("q c -> c q"))
                nc.sync.dma_start(out=oxyz, in_=goal_h[b].rearrange("q c -> c q"))
            nc.vector.tensor_tensor(out=dispt, in0=oxyz, in1=gxyz, op=OP.subtract)
            nc.sync.dma_start(out=in6[0][0:3], in_=gxyz)
            nc.sync.dma_start(out=in6[0][3:6], in_=dispt)
            nc.sync.dma_start(out=in6[1][0:3], in_=oxyz)
            nc.sync.dma_start(out=in6[1][3:6], in_=dispt)

            for m in range(2):
                # query rope tables
                aqp = psum.tile([PAIRS, NGP], f32, tag="ps")
                nc.tensor.matmul(aqp, lhsT=W["msel"][:], rhs=in6[m][0:3],
                                 start=True, stop=True)
                sinq[m] = qs.tile([PAIRS, NGP], f32, tag=f"sinq{m}", name=f"sinq{m}")
                cosq[m] = qs.tile([PAIRS, NGP], f32, tag=f"cosq{m}", name=f"cosq{m}")
                nc.scalar.activation(sinq[m], aqp, AF.Sin)
                nc.scalar.activation(cosq[m], aqp, AF.Sin, bias=pihalf[:PAIRS])
                # gripper PE MLP
                g1p = psum.tile([128, NGP], f32, tag="ps")
                nc.tensor.matmul(g1p, lhsT=W["gw1"][:, m], rhs=in6[m],
                                 start=True, stop=True)
                g1 = qs.tile([128, NGP], f32, tag="g1")
                nc.scalar.activation(g1, g1p, AF.Relu, bias=W["gb1"][:, m:m + 1])
                g2 = qs.tile([128, 2, NGP], f32, tag="g2")
                for oc in range(2):
                    g2p = psum.tile([128, NGP], f32, tag="ps")
                    nc.tensor.matmul(g2p, lhsT=W["gw2"][:, m, oc * 128:(oc + 1) * 128],
                                     rhs=g1, start=True, stop=True)
                    nc.scalar.activation(g2[:, oc], g2p, AF.Relu,
                                         bias=W["gb2"][:, 2 * m + oc:2 * m + oc + 1])
                g3p = psum.tile([80, NGP], f32, tag="ps")
                for cc in range(2):
                    nc.tensor.matmul(g3p, lhsT=W["gw3"][:, m, cc], rhs=g2[:, cc],
                                     start=(cc == 0), stop=(cc == 1))
                gpeT = qs.tile([80, NGP], f32, tag="gpeT")
                nc.vector.tensor_scalar_add(gpeT, g3p, W["gb3"][:, m:m + 1])
                gperp = psum.tile([NGP, 80], f32, tag="ps")
                nc.tensor.transpose(gperp, gpeT, ident[:80, :80])
                gf = qs.tile([NGP, 240], f32, tag=f"x_{m}")
                nc.vector.tensor_copy(gf[:, 0:160], emb_s[:, m])
                nc.vector.tensor_copy(gf[:, 160:240], gperp)
                xstate[m] = gf

            # ======== attention layers ========
            for l in range(2):
                blkE, blkO, blkEn = {}, {}, {}
                for m in range(2):
                    lm = m * 2 + l
                    x = xstate[m]
                    xT = qs.tile([120, 2, NGP], f32, tag=f"xT{m}")
                    for cc in range(2):
                        xtp = psum.tile([120, NGP], f32, tag="ps")
                        nc.tensor.transpose(xtp, x[:, cc * 120:(cc + 1) * 120],
                                            ident[:NGP, :NGP])
                        nc.vector.tensor_copy(xT[:, cc], xtp)
                    qsb = qs.tile([120, 2, NGP], f32, tag=f"qsb{m}")
                    for oc in range(2):
                        qp = psum.tile([120, NGP], f32, tag="ps")
                        for cc in range(2):
                            nc.tensor.matmul(
                                qp, lhsT=W["wq"][:, lm, cc, oc * 120:(oc + 1) * 120],
                                rhs=xT[:, cc], start=(cc == 0), stop=(cc == 1))
                        nc.vector.tensor_scalar_add(qsb[:, oc], qp,
                                                    W["bq"][:, lm, oc:oc + 1])
                    t1 = qs.tile([120, NGP], f32, tag="t1")
                    t2 = qs.tile([120, NGP], f32, tag="t2")
                    qrE = qs.tile([120, NGP], f32r, tag=f"qrE{m}")
                    qrO = qs.tile([120, NGP], f32r, tag=f"qrO{m}")
                    nc.vector.tensor_tensor(out=t1, in0=qsb[:, 0], in1=cosq[m], op=OP.mult)
                    nc.vector.tensor_tensor(out=t2, in0=qsb[:, 1], in1=sinq[m], op=OP.mult)
                    nc.vector.tensor_tensor(out=qrE, in0=t1, in1=t2, op=OP.subtract)
                    nc.vector.tensor_tensor(out=t1, in0=qsb[:, 1], in1=cosq[m], op=OP.mult)
                    nc.vector.tensor_tensor(out=t2, in0=qsb[:, 0], in1=sinq[m], op=OP.mult)
                    nc.vector.tensor_tensor(out=qrO, in0=t1, in1=t2, op=OP.add)
                    bE = qs.tile([120, 12], f32r, tag=f"blkE{m}")
                    bO = qs.tile([120, 12], f32r, tag=f"blkO{m}")
                    bEn = qs.tile([120, 12], f32r, tag=f"blkEn{m}")
                    nc.sync.dma_start(out=bE, in_=zblk.bitcast(f32r))
                    nc.sync.dma_start(out=bO, in_=zblk.bitcast(f32r))
                    for h in range(3):
                        nc.sync.dma_start(
                            out=bE[40 * h:40 * (h + 1), 4 * h:4 * (h + 1)],
                            in_=qrE[40 * h:40 * (h + 1), :])
                        nc.sync.dma_start(
                            out=bO[40 * h:40 * (h + 1), 4 * h:4 * (h + 1)],
                            in_=qrO[40 * h:40 * (h + 1), :])
                    nc.vector.tensor_scalar_mul(bEn, bE, -1.0)
                    blkE[m], blkO[m], blkEn[m] = bE, bO, bEn

                Zbuf = {m: qs.tile([12, NT], f32, tag=f"Z{m}", name=f"Zbuf{m}") for m in range(2)}
                afp = {m: psum.tile([12, 240], f32, tag="af", name=f"afp{m}") for m in range(2)}

                for t in range(NT):
                    n0 = t * T
                    for m in range(2):
                        lm = m * 2 + l
                        kps = [psum.tile([120, T], f32, tag="K", name=f"kps{oc_}") for oc_ in range(2)]
                        for oc in range(2):
                            for cc in range(2):
                                nc.tensor.matmul(
                                    kps[oc],
                                    lhsT=W["wk"][:, lm, cc, oc * 120:(oc + 1) * 120],
                                    rhs=featT[cc][:, n0:n0 + T],
                                    start=(cc == 0), stop=(cc == 1))
                        CKe = ck.tile([120, T], f32r, tag="CKe")
                        CKo = ck.tile([120, T], f32r, tag="CKo")
                        SKe = ck.tile([120, T], f32r, tag="SKe")
                        SKo = ck.tile([120, T], f32r, tag="SKo")
                        for out_t, kp, bslice, trig in (
                                (CKe, kps[0], 0, cosP), (SKe, kps[0], 0, sinP),
                                (CKo, kps[1], 1, cosP), (SKo, kps[1], 1, sinP)):
                            nc.vector.scalar_tensor_tensor(
                                out=out_t, in0=kp, scalar=W["bk"][:, lm, bslice:bslice + 1],
                                in1=trig[:, n0:n0 + T], op0=OP.add, op1=OP.mult)
                        sps = psum.tile([12, T], f32, tag="ps")
                        nc.tensor.matmul(sps, lhsT=blkE[m], rhs=CKe,
                                         start=True, stop=False)
                        nc.tensor.matmul(sps, lhsT=blkO[m], rhs=CKo,
                                         start=False, stop=False)
                        nc.tensor.matmul(sps, lhsT=blkO[m], rhs=SKe,
                                         start=False, stop=False)
                        nc.tensor.matmul(sps, lhsT=blkEn[m], rhs=SKo,
                                         start=False, stop=True)
                        expS = sb.tile([12, T], f32, tag="expS")
                        nc.scalar.activation(expS, sps, AF.Exp,
                                             accum_out=Zbuf[m][:, t:t + 1])
                        for j in range(4):
                            etp = psum.tile([128, 12], f32, tag="ps")
                            nc.tensor.transpose(etp, expS[:, j * 128:(j + 1) * 128],
                                                ident[:12, :12])
                            eT = sb.tile([128, 12], bf16, tag="eT", bufs=4)
                            nc.scalar.copy(eT, etp)
                            nc.tensor.matmul(afp[m], lhsT=eT, rhs=frow[:, 4 * t + j],
                                             start=(t == 0 and j == 0),
                                             stop=(t == NT - 1 and j == 3))

                # ---- per-module epilogue: normalize, project, LN ----
                for m in range(2):
                    lm = m * 2 + l
                    Zs = qs.tile([12, 1], f32, tag="Zs")
                    nc.vector.tensor_reduce(out=Zs, in_=Zbuf[m], axis=AX.X, op=OP.add)
                    Zr = qs.tile([12, 1], f32, tag="Zr")
                    nc.vector.reciprocal(out=Zr, in_=Zs)
                    af = qs.tile([12, 240], f32, tag="af_sb")
                    nc.vector.tensor_scalar_mul(af, afp[m], Zr)
                    afT = qs.tile([120, 2, 12], f32, tag="afT")
                    for dc in range(2):
                        atp = psum.tile([120, 12], f32, tag="ps")
                        nc.tensor.transpose(atp, af[:, dc * 120:(dc + 1) * 120],
                                            ident[:12, :12])
                        nc.vector.tensor_copy(afT[:, dc], atp)
                    avsb = qs.tile([120, 2, 12], f32, tag="avsb")
                    for dc in range(2):
                        avp = psum.tile([120, 12], f32, tag="ps")
                        for cc in range(2):
                            nc.tensor.matmul(
                                avp, lhsT=W["wv"][:, lm, cc, dc * 120:(dc + 1) * 120],
                                rhs=afT[:, cc], start=(cc == 0), stop=(cc == 1))
                        nc.vector.tensor_scalar_add(avsb[:, dc], avp,
                                                    W["bv"][:, lm, dc:dc + 1])
                    avsel = qs.tile([120, 2, NGP], f32, tag="avsel")
                    nc.sync.dma_start(out=avsel[0:80, 0], in_=avsb[0:80, 0, 0:4])
                    nc.sync.dma_start(out=avsel[80:120, 0], in_=avsb[80:120, 0, 4:8])
                    nc.sync.dma_start(out=avsel[0:40, 1], in_=avsb[0:40, 1, 4:8])
                    nc.sync.dma_start(out=avsel[40:120, 1], in_=avsb[40:120, 1, 8:12])
                    osp = psum.tile([NGP, 240], f32, tag="ps")
                    for dc in range(2):
                        nc.tensor.matmul(osp, lhsT=avsel[:, dc], rhs=W["wo"][:, lm, dc],
                                         start=(dc == 0), stop=(dc == 1))
                    gbt = qs.tile([NGP, 3, 240], f32, tag="gbt")
                    nc.gpsimd.dma_start(out=gbt, in_=bcast(w_h["gbb"][lm], NGP))
                    xn = qs.tile([NGP, 240], f32, tag=f"x_{m}")
                    nc.vector.tensor_tensor(out=xn, in0=osp, in1=xstate[m], op=OP.add)
                    nc.vector.tensor_tensor(out=xn, in0=xn, in1=gbt[:, 0], op=OP.add)
                    # LayerNorm
                    stats = qs.tile([NGP, 6], f32, tag="stats")
                    nc.vector.bn_stats(out=stats, in_=xn)
                    mv = qs.tile([NGP, 2], f32, tag="mv")
                    nc.vector.bn_aggr(out=mv, in_=stats)
                    lnv = qs.tile([NGP, 1], f32, tag="lnv")
                    nc.scalar.activation(lnv, mv[:, 1:2], AF.Ln, bias=epst)
                    rstd = qs.tile([NGP, 1], f32, tag="rstd")
                    nc.scalar.activation(rstd, lnv, AF.Exp, scale=-0.5)
                    nc.vector.tensor_scalar(out=xn, in0=xn, scalar1=mv[:, 0:1],
                                            scalar2=rstd, op0=OP.subtract, op1=OP.mult)
                    nc.vector.tensor_tensor(out=xn, in0=xn, in1=gbt[:, 1], op=OP.mult)
                    nc.vector.tensor_tensor(out=xn, in0=xn, in1=gbt[:, 2], op=OP.add)
                    xstate[m] = xn

            # ---- write outputs for batch b ----
            nc.sync.dma_start(
                out=out_h[b, 0:NGP * D].rearrange("(q d) -> q d", q=NGP),
                in_=xstate[0])
            nc.sync.dma_start(
                out=out_h[b, NGP * D + 64:OUTF].rearrange("(q d) -> q d", q=NGP),
                in_=xstate[1])

    nc.finalize()
    return nc


_CACHE = {}


def kernel(point_cloud, gripper_pcd, goal_gripper_pcd, agent_pos, params):
    from concourse.bass_utils import run_bass_kernel_spmd

    point_cloud = np.ascontiguousarray(point_cloud, F32)
    gripper_pcd = np.ascontiguousarray(gripper_pcd, F32)
    goal_gripper_pcd = np.ascontiguousarray(goal_gripper_pcd, F32)
    agent_pos = np.ascontiguousarray(agent_pos, F32)

    consts = _prep_consts(params)
    if "nc" not in _CACHE:
        _CACHE["nc"] = _build_program(consts)
    nc = _CACHE["nc"]

    in_maps = []
    for c in range(NC_COUNT):
        s = slice(c * BL, (c + 1) * BL)
        m = {"pc": point_cloud[s], "grip": gripper_pcd[s],
             "goal": goal_gripper_pcd[s], "agent": agent_pos[s]}
        for name, arr in consts.items():
            m[name] = arr
        in_maps.append(m)

    res = run_bass_kernel_spmd(nc, in_maps, list(range(NC_COUNT)))
    return np.concatenate([r["out"] for r in res.results], axis=0)


# revision 27
# speedup vs baseline: 1609.8519x; 1609.8519x over previous
"""Trainium2 Bass kernel for nn_Act3dEncoder (dense_transformer).

Sharding: data-parallel over batch B=32 across 8 NeuronCores (4 batches/core),
params replicated. All compute on device. Key algorithmic restructurings
(validated against the reference to ~7e-7 in fp32):

 - Activations kept feature-transposed ([feat, n] on partitions) so every GEMM
   contracts on partitions with the weights stationary.
 - Feature dims de-interleaved (evens|odds) via host-side row permutation of
   Wq/Wk so RoPE pair structure becomes two aligned [120, n] chunks.
 - Rotation folded into the PSUM evacuation: CK/SK = (K + bk) * cos/sin via
   one scalar_tensor_tensor pass each; scores computed as a 4-chunk augmented
   contraction against head-blocked rotated queries (zeros off-head), which
   also realizes the per-head block-diagonal attention in one matmul.
 - attn@V collapsed via attnF: af = softmax-unnormalized(exp(s)) @ feat, then
   av = (af/Z) @ Wv^T + bv (softmax rows sum to 1). Eliminates all V
   projections over N entirely.
 - Scores are small (|s| < 0.5 for this model), so exp() without
   max-subtraction; Z accumulated for free via the ACT accum_out port.
 - fp32r (full-rate) for the big matmuls, bf16 for cos/sin tables, featrow and
   exp(scores); everything else fp32.
"""

import sys

sys.path.insert(0, "/opt/trn_rl_repo")

import numpy as np
import ml_dtypes

B, N, D, H, NGP = 32, 8192, 240, 3, 4
HD = D // H          # 80
NC_COUNT = 8
BL = B // NC_COUNT   # 4 batches per core
PAIRS = D // 2       # 120
T = 512              # n-tile
NT = N // T          # 16
OUTF = NGP * D + 64 + NGP * D  # 1984
F32 = np.float32

_PERM = np.concatenate([np.arange(0, D, 2), np.arange(1, D, 2)])  # evens|odds


def _prep_consts(params):
    """Host-side weight layout prep. Returns dict name -> np.ndarray."""
    c = {}
    vis = params["vision"]
    # vision layer 1: W [240, 3] -> lhsT [3, 240]
    c["w1t"] = np.ascontiguousarray(vis[0]["W"].T, F32)  # [3, 240]
    for i, nm in ((1, "w2t"), (2, "w3t")):
        wt = vis[i]["W"].T.astype(F32)  # [240 in, 240 out]
        c[nm] = np.ascontiguousarray(
            wt.reshape(2, 120, 240).transpose(1, 0, 2))  # [cp, cc, out]
    vb = np.stack([vis[i]["b"].astype(F32) for i in range(3)], -1)  # [240, 3]
    c["vb"] = np.ascontiguousarray(vb.reshape(2, 120, 3).transpose(1, 0, 2))  # [120,2,3]

    # RoPE frequency selection matrix: ang[p, n] = sum_a msel[a, p] * xyz[a, n]
    d3 = D // 3
    div = np.exp(np.arange(0, d3, 2, dtype=F32) * (-np.log(10000.0) / d3))  # [40]
    msel = np.zeros((3, PAIRS), F32)
    for p in range(PAIRS):
        msel[p // 40, p] = div[p % 40]
    c["msel"] = msel
    c["msel_r"] = msel.copy()

    # attention weights: lm = module*2 + layer
    wq = np.zeros((120, 4, 2, 240), F32)
    wk = np.zeros((121, 4, 2, 240), F32)
    wv = np.zeros((120, 4, 2, 240), F32)
    wo = np.zeros((120, 4, 2, 240), F32)
    bq = np.zeros((120, 4, 2), F32)
    bk = np.zeros((120, 4, 2), F32)
    bv = np.zeros((120, 4, 2), F32)
    gbb = np.zeros((4, 3, 240), F32)  # [lm, (bo,g,beta), d]
    scale = HD ** -0.5
    for m, key in ((0, "attn"), (1, "goal_attn")):
        for l, p in enumerate(params[key]):
            lm = m * 2 + l
            Wq, Wk, Wv = np.split(np.asarray(p["Wqkv"], F32), 3, axis=0)
            bq_, bk_, bv_ = np.split(np.asarray(p["bqkv"], F32), 3)
            wqt = (Wq * scale).T[:, _PERM]   # [240c, 240 out-permuted]
            wkt = Wk.T[:, _PERM]
            wvt = Wv.T                        # natural out
            wot = np.asarray(p["Wo"], F32).T  # [240 in(avsel d), 240 out]
            for arr, w in ((wq, wqt), (wv, wvt), (wo, wot)):
                arr[:, lm] = w.reshape(2, 120, 240).transpose(1, 0, 2)
            wk[0:120, lm] = wkt.reshape(2, 120, 240).transpose(1, 0, 2)
            wk[120, lm, 0] = bk_[_PERM]  # bias via constant-ones feature row
            bq[:, lm] = (bq_ * scale)[_PERM].reshape(2, 120).T
            bk[:, lm] = bk_[_PERM].reshape(2, 120).T
            bv[:, lm] = bv_.reshape(2, 120).T
            gbb[lm, 0] = p["bo"]
            gbb[lm, 1] = p["g"]
            gbb[lm, 2] = p["beta"]
    c["wq"], c["wv"], c["wo"] = wq, wv, wo
    c["wk"] = wk.astype(ml_dtypes.bfloat16)
    c["bq"], c["bk"], c["bv"], c["gbb"] = bq, bk, bv, gbb

    # gripper / goal PE MLPs [6 -> 128 -> 256 -> 80]
    gw1 = np.zeros((6, 2, 128), F32)
    gb1 = np.zeros((128, 2), F32)
    gw2 = np.zeros((128, 2, 256), F32)
    gb2 = np.zeros((128, 4), F32)   # [p, which*2 + oc]
    gw3 = np.zeros((128, 2, 2, 80), F32)
    gb3 = np.zeros((80, 2), F32)
    for w, key in ((0, "gripper_pe_mlp"), (1, "goal_pe_mlp")):
        ps = params[key]
        gw1[:, w] = np.asarray(ps[0]["W"], F32).T
        gb1[:, w] = ps[0]["b"]
        gw2[:, w] = np.asarray(ps[1]["W"], F32).T
        gb2[:, w * 2 + 0] = ps[1]["b"][:128]
        gb2[:, w * 2 + 1] = ps[1]["b"][128:]
        w3t = np.asarray(ps[2]["W"], F32).T  # [256, 80]
        gw3[:, w, 0] = w3t[:128]
        gw3[:, w, 1] = w3t[128:]
        gb3[:, w] = ps[2]["b"]
    c["gw1"], c["gb1"], c["gw2"], c["gb2"] = gw1, gb1, gw2, gb2
    c["gw3"], c["gb3"] = gw3, gb3
    c["embeds"] = np.stack([np.asarray(params["embed"], F32),
                            np.asarray(params["goal_embed"], F32)])  # [2, 160]

    st = params["state_mlp"]
    c["sw1"] = np.ascontiguousarray(np.asarray(st[0]["W"], F32).T)  # [8, 64]
    c["sw2"] = np.ascontiguousarray(np.asarray(st[1]["W"], F32).T)  # [64, 64]
    c["sb"] = np.stack([np.asarray(st[0]["b"], F32),
                        np.asarray(st[1]["b"], F32)], -1)  # [64, 2]
    return c


def _build_program(consts):
    import ml_dtypes
    import concourse.bass as bass
    import concourse.bacc as bacc
    import concourse.mybir as mybir
    import concourse.tile as tile
    from concourse.masks import make_identity
    from contextlib import ExitStack

    f32 = mybir.dt.float32
    f32r = mybir.dt.float32r
    bf16 = mybir.dt.bfloat16
    AF = mybir.ActivationFunctionType
    OP = mybir.AluOpType
    AX = mybir.AxisListType

    nc = bacc.Bacc(None, target_bir_lowering=False)

    pc_h = nc.declare_dram_parameter("pc", [BL, N, 3], f32, isOutput=False)
    grip_h = nc.declare_dram_parameter("grip", [BL, NGP, 3], f32, isOutput=False)
    goal_h = nc.declare_dram_parameter("goal", [BL, NGP, 3], f32, isOutput=False)
    agent_h = nc.declare_dram_parameter("agent", [BL, 8], f32, isOutput=False)
    F32R_W = {"w1t", "w2t", "w3t", "msel_r"}
    w_h = {}
    for name, arr in consts.items():
        if arr.dtype == ml_dtypes.bfloat16:
            dt_ = bf16
        elif name in F32R_W:
            dt_ = f32r
        else:
            dt_ = f32
        w_h[name] = nc.declare_dram_parameter(name, list(arr.shape), dt_,
                                              isOutput=False)
    out_h = nc.declare_dram_parameter("out", [BL, OUTF], f32, isOutput=True)

    def bcast(handle, p):
        ap = handle[:]
        return bass.AP(tensor=ap.tensor, offset=ap.offset,
                       ap=[[0, p]] + list(ap.ap))

    def r(ap):
        return ap.bitcast(f32r)

    with tile.TileContext(nc) as tc, ExitStack() as ctx:
        wp = ctx.enter_context(tc.tile_pool(name="wp", bufs=1))
        persist = ctx.enter_context(tc.tile_pool(name="persist", bufs=1))
        sb = ctx.enter_context(tc.tile_pool(name="sb", bufs=2))
        ck = ctx.enter_context(tc.tile_pool(name="ck", bufs=2))
        qs = ctx.enter_context(tc.tile_pool(name="qs", bufs=2))
        psum = ctx.enter_context(tc.tile_pool(name="psum", bufs=2, space="PSUM"))

        # ---- load weights to SBUF ----
        W = {}
        for name, arr in consts.items():
            if arr.dtype == ml_dtypes.bfloat16:
                dt_ = bf16
            elif name in F32R_W:
                dt_ = f32r
            else:
                dt_ = f32
            tile_ = wp.tile(list(arr.shape), dt_, tag=f"w_{name}", name=f"w_{name}")
            nc.sync.dma_start(out=tile_, in_=w_h[name][:])
            W[name] = tile_
        ident = wp.tile([128, 128], f32, tag="ident")
        make_identity(nc, ident)
        ident_bf = wp.tile([128, 128], bf16, tag="ident_bf")
        make_identity(nc, ident_bf)
        pihalf = wp.tile([128, 1], f32, tag="pihalf")
        nc.vector.memset(pihalf, float(np.pi / 2))
        epst = wp.tile([NGP, 1], f32, tag="epst")
        nc.vector.memset(epst, 1e-5)
        zblk = wp.tile([120, 12], bf16, tag="zblk")
        nc.vector.memset(zblk, 0.0)
        # broadcast constants over rows
        emb_s = wp.tile([NGP, 2, 160], f32, tag="emb_s")
        nc.gpsimd.dma_start(out=emb_s, in_=bcast(w_h["embeds"], NGP))

        # ---- state MLP (all 4 batches at once) ----
        sT = qs.tile([8, BL], f32, tag="sT")
        with nc.allow_non_contiguous_dma(reason="tiny transposed load"):
            nc.sync.dma_start(out=sT, in_=agent_h[:].rearrange("b f -> f b"))
        s1p = psum.tile([64, BL], f32, tag="ps", bufs=3)
        nc.tensor.matmul(s1p, lhsT=W["sw1"][:], rhs=sT, start=True, stop=True)
        s1 = qs.tile([64, BL], f32, tag="s1")
        nc.scalar.activation(s1, s1p, AF.Relu, bias=W["sb"][:, 0:1])
        s2p = psum.tile([64, BL], f32, tag="ps", bufs=3)
        nc.tensor.matmul(s2p, lhsT=W["sw2"][:], rhs=s1, start=True, stop=True)
        s2 = qs.tile([64, BL], f32, tag="s2")
        nc.vector.tensor_scalar_add(s2, s2p, W["sb"][:, 1:2])
        strp = psum.tile([BL, 64], f32, tag="ps", bufs=3)
        nc.tensor.transpose(strp, s2, ident[:64, :64])
        strow = qs.tile([BL, 64], f32, tag="strow")
        nc.vector.tensor_copy(strow, strp)
        nc.sync.dma_start(out=out_h[:, NGP * D:NGP * D + 64], in_=strow)

        featT = [persist.tile([128, N], bf16, tag=f"featT{cc}", name=f"featT{cc}")
                 for cc in range(2)]
        nc.vector.memset(featT[0][96:128, :], 1.0)  # row 120 = bias ones; 96:120 overwritten by L3

        for b in range(BL):
            # ======== vision + rope tables for batch b ========
            cosP = persist.tile([PAIRS, N], bf16, tag="cosP")
            sinP = persist.tile([PAIRS, N], bf16, tag="sinP")
            frow = persist.tile([128, N // 128, 256], bf16, tag="frow")

            pct = persist.tile([128, N // 128, 3], f32, tag="pct")
            nc.sync.dma_start(
                out=pct, in_=pc_h[b].rearrange("(cb p) c -> p cb c", p=128))

            for t in range(NT):
                n0 = t * T
                xyzT = sb.tile([3, T], f32r, tag="xyzT")
                for j in range(4):
                    cb = t * 4 + j
                    tp = psum.tile([3, 128], f32, tag="ps", bufs=3)
                    nc.tensor.transpose(tp, pct[:, cb, :], ident)
                    nc.vector.tensor_copy(xyzT[:, j * 128:(j + 1) * 128], tp)
                # rope angles -> cos/sin
                angp = psum.tile([PAIRS, T], f32, tag="af", bufs=1)
                nc.tensor.matmul(angp, lhsT=W["msel_r"][:], rhs=xyzT,
                                 start=True, stop=True)
                nc.scalar.activation(sinP[:, n0:n0 + T], angp, AF.Sin)
                nc.scalar.activation(cosP[:, n0:n0 + T], angp, AF.Sin,
                                     bias=pihalf[:PAIRS])
                # vision MLP (each layer: one 2-bank psum tile, 2 matmuls, 2 evacs)
                x1 = sb.tile([120, 2, T], f32r, tag="x1")
                p1 = psum.tile([120, 2, T], f32, tag="K", bufs=2)
                for oc in range(2):
                    nc.tensor.matmul(p1[:, oc], lhsT=W["w1t"][:, oc * 120:(oc + 1) * 120],
                                     rhs=xyzT, start=True, stop=True)
                for oc in range(2):
                    nc.vector.tensor_scalar(
                        out=x1[:, oc], in0=p1[:, oc], scalar1=W["vb"][:, oc, 0:1],
                        scalar2=0.0, op0=OP.add, op1=OP.max)
                x2 = sb.tile([120, 2, T], f32r, tag="x2")
                p2 = psum.tile([120, 2, T], f32, tag="K", bufs=2)
                for oc in range(2):
                    for cc in range(2):
                        nc.tensor.matmul(
                            p2[:, oc], lhsT=W["w2t"][:, cc, oc * 120:(oc + 1) * 120],
                            rhs=x1[:, cc], start=(cc == 0), stop=(cc == 1))
                for oc in range(2):
                    nc.scalar.activation(x2[:, oc], p2[:, oc], AF.Relu,
                                         bias=W["vb"][:, oc, 1:2])
                p3 = psum.tile([120, 2, T], f32, tag="K", bufs=2)
                for oc in range(2):
                    for cc in range(2):
                        nc.tensor.matmul(
                            p3[:, oc], lhsT=W["w3t"][:, cc, oc * 120:(oc + 1) * 120],
                            rhs=x2[:, cc], start=(cc == 0), stop=(cc == 1))
                nc.vector.tensor_scalar_add(featT[0][0:120, n0:n0 + T], p3[:, 0],
                                            W["vb"][:, 0, 2:3])
                nc.scalar.activation(featT[1][0:120, n0:n0 + T], p3[:, 1], AF.Identity,
                                     bias=W["vb"][:, 1, 2:3])
            # featrow via PE transposes (xbar DMA transpose hangs HW here)
            for cb in range(N // 128):
                for cc in range(2):
                    fp = psum.tile([128, 120], bf16, tag="ps", bufs=3, name="fp")
                    nc.tensor.transpose(fp, featT[cc][0:120, cb * 128:(cb + 1) * 128],
                                        ident_bf[:120, :120])
                    if (cb + cc) % 2 == 0:
                        nc.scalar.copy(frow[:, cb, cc * 128:cc * 128 + 120], fp)
                    else:
                        nc.vector.tensor_copy(frow[:, cb, cc * 128:cc * 128 + 120], fp)

            # ======== gripper-side module inputs ========
            in6 = {}
            cosq, sinq = {}, {}
            xstate = {}
            for m in range(2):
                in6[m] = qs.tile([6, NGP], f32, tag=f"in6_{m}", name=f"in6_{m}")
            gxyz = qs.tile([3, NGP], f32, tag="gxyz")
            oxyz = qs.tile([3, NGP], f32, tag="oxyz")
            dispt = qs.tile([3, NGP], f32, tag="dispt")
            with nc.allow_non_contiguous_dma(reason="tiny transposed load"):
                nc.sync.dma_start(out=gxyz, in_=grip_h[b].rearrange("q c -> c q"))
                nc.sync.dma_start(out=oxyz, in_=goal_h[b].rearrange("q c -> c q"))
            nc.vector.tensor_tensor(out=dispt, in0=oxyz, in1=gxyz, op=OP.subtract)
            nc.sync.dma_start(out=in6[0][0:3], in_=gxyz)
            nc.sync.dma_start(out=in6[0][3:6], in_=dispt)
            nc.sync.dma_start(out=in6[1][0:3], in_=oxyz)
            nc.sync.dma_start(out=in6[1][3:6], in_=dispt)

            for m in range(2):
                # query rope tables
                aqp = psum.tile([PAIRS, NGP], f32, tag="ps", bufs=3)
                nc.tensor.matmul(aqp, lhsT=W["msel"][:], rhs=in6[m][0:3],
                                 start=True, stop=True)
                sinq[m] = qs.tile([PAIRS, NGP], f32, tag=f"sinq{m}", name=f"sinq{m}")
                cosq[m] = qs.tile([PAIRS, NGP], f32, tag=f"cosq{m}", name=f"cosq{m}")
                nc.scalar.activation(sinq[m], aqp, AF.Sin)
                nc.scalar.activation(cosq[m], aqp, AF.Sin, bias=pihalf[:PAIRS])
                # gripper PE MLP
                g1p = psum.tile([128, NGP], f32, tag="ps", bufs=3)
                nc.tensor.matmul(g1p, lhsT=W["gw1"][:, m], rhs=in6[m],
                                 start=True, stop=True)
                g1 = qs.tile([128, NGP], f32, tag="g1")
                nc.scalar.activation(g1, g1p, AF.Relu, bias=W["gb1"][:, m:m + 1])
                g2 = qs.tile([128, 2, NGP], f32, tag="g2")
                for oc in range(2):
                    g2p = psum.tile([128, NGP], f32, tag="ps", bufs=3)
                    nc.tensor.matmul(g2p, lhsT=W["gw2"][:, m, oc * 128:(oc + 1) * 128],
                                     rhs=g1, start=True, stop=True)
                    nc.scalar.activation(g2[:, oc], g2p, AF.Relu,
                                         bias=W["gb2"][:, 2 * m + oc:2 * m + oc + 1])
                g3p = psum.tile([80, NGP], f32, tag="ps", bufs=3)
                for cc in range(2):
                    nc.tensor.matmul(g3p, lhsT=W["gw3"][:, m, cc], rhs=g2[:, cc],
                                     start=(cc == 0), stop=(cc == 1))
                gpeT = qs.tile([80, NGP], f32, tag="gpeT")
                nc.vector.tensor_scalar_add(gpeT, g3p, W["gb3"][:, m:m + 1])
                gperp = psum.tile([NGP, 80], f32, tag="ps", bufs=3)
                nc.tensor.transpose(gperp, gpeT, ident[:80, :80])
                gf = qs.tile([NGP, 240], f32, tag=f"x_{m}")
                nc.vector.tensor_copy(gf[:, 0:160], emb_s[:, m])
                nc.vector.tensor_copy(gf[:, 160:240], gperp)
                xstate[m] = gf

            # ======== attention layers ========
            for l in range(2):
                blkE, blkO, blkEn = {}, {}, {}
                for m in range(2):
                    lm = m * 2 + l
                    x = xstate[m]
                    xT = qs.tile([120, 2, NGP], f32, tag=f"xT{m}")
                    for cc in range(2):
                        xtp = psum.tile([120, NGP], f32, tag="ps", bufs=3)
                        nc.tensor.transpose(xtp, x[:, cc * 120:(cc + 1) * 120],
                                            ident[:NGP, :NGP])
                        nc.vector.tensor_copy(xT[:, cc], xtp)
                    qsb = qs.tile([120, 2, NGP], f32, tag=f"qsb{m}")
                    for oc in range(2):
                        qp = psum.tile([120, NGP], f32, tag="ps", bufs=3)
                        for cc in range(2):
                            nc.tensor.matmul(
                                qp, lhsT=W["wq"][:, lm, cc, oc * 120:(oc + 1) * 120],
                                rhs=xT[:, cc], start=(cc == 0), stop=(cc == 1))
                        nc.vector.tensor_scalar_add(qsb[:, oc], qp,
                                                    W["bq"][:, lm, oc:oc + 1])
                    t1 = qs.tile([120, NGP], f32, tag="t1")
                    t2 = qs.tile([120, NGP], f32, tag="t2")
                    qrE = qs.tile([120, NGP], bf16, tag=f"qrE{m}")
                    qrO = qs.tile([120, NGP], bf16, tag=f"qrO{m}")
                    nc.vector.tensor_tensor(out=t1, in0=qsb[:, 0], in1=cosq[m], op=OP.mult)
                    nc.vector.tensor_tensor(out=t2, in0=qsb[:, 1], in1=sinq[m], op=OP.mult)
                    nc.vector.tensor_tensor(out=qrE, in0=t1, in1=t2, op=OP.subtract)
                    nc.vector.tensor_tensor(out=t1, in0=qsb[:, 1], in1=cosq[m], op=OP.mult)
                    nc.vector.tensor_tensor(out=t2, in0=qsb[:, 0], in1=sinq[m], op=OP.mult)
                    nc.vector.tensor_tensor(out=qrO, in0=t1, in1=t2, op=OP.add)
                    bE = qs.tile([120, 12], bf16, tag=f"blkE{m}")
                    bO = qs.tile([120, 12], bf16, tag=f"blkO{m}")
                    bEn = qs.tile([120, 12], bf16, tag=f"blkEn{m}")
                    nc.sync.dma_start(out=bE, in_=zblk)
                    nc.sync.dma_start(out=bO, in_=zblk)
                    for h in range(3):
                        nc.sync.dma_start(
                            out=bE[40 * h:40 * (h + 1), 4 * h:4 * (h + 1)],
                            in_=qrE[40 * h:40 * (h + 1), :])
                        nc.sync.dma_start(
                            out=bO[40 * h:40 * (h + 1), 4 * h:4 * (h + 1)],
                            in_=qrO[40 * h:40 * (h + 1), :])
                    nc.vector.tensor_scalar_mul(bEn, bE, -1.0)
                    blkE[m], blkO[m], blkEn[m] = bE, bO, bEn

                Zbuf = qs.tile([44, NT], f32, tag="Zbuf", name="Zbuf")
                # merged A+G af accumulator: rows 0:12 = module A, 32:44 = G
                afp = psum.tile([44, 256], f32, tag="af", name="afp", bufs=1)

                for t in range(NT):
                    n0 = t * T
                    expST = sb.tile([44, T], bf16, tag="expST")
                    for m in range(2):
                        lm = m * 2 + l
                        kps = psum.tile([120, 2, T], f32, tag="K", name="kps", bufs=2)
                        for oc in range(2):
                            nc.tensor.matmul(
                                kps[:, oc],
                                lhsT=W["wk"][0:121, lm, 0, oc * 120:(oc + 1) * 120],
                                rhs=featT[0][0:121, n0:n0 + T],
                                start=True, stop=False)
                            nc.tensor.matmul(
                                kps[:, oc],
                                lhsT=W["wk"][0:120, lm, 1, oc * 120:(oc + 1) * 120],
                                rhs=featT[1][0:120, n0:n0 + T],
                                start=False, stop=True)
                        kb = sb.tile([120, 2, T], bf16, tag="kb")
                        nc.scalar.copy(kb, kps)
                        CK = ck.tile([120, 2, T], bf16, tag="CK")
                        SK = ck.tile([120, 2, T], bf16, tag="SK")
                        cp = cosP[:, n0:n0 + T]
                        sp_ = sinP[:, n0:n0 + T]
                        cpb = bass.AP(tensor=cp.tensor, offset=cp.offset,
                                      ap=[list(cp.ap[0]), [0, 2], list(cp.ap[1])])
                        spb = bass.AP(tensor=sp_.tensor, offset=sp_.offset,
                                      ap=[list(sp_.ap[0]), [0, 2], list(sp_.ap[1])])
                        nc.vector.tensor_tensor(out=CK, in0=kb, in1=cpb, op=OP.mult)
                        nc.vector.tensor_tensor(out=SK, in0=kb, in1=spb, op=OP.mult)
                        so = psum.tile([12, T], f32, tag="ps", bufs=3, name="so")
                        nc.tensor.matmul(so, lhsT=blkE[m], rhs=CK[:, 0],
                                         start=True, stop=False)
                        nc.tensor.matmul(so, lhsT=blkO[m], rhs=CK[:, 1],
                                         start=False, stop=False)
                        nc.tensor.matmul(so, lhsT=blkO[m], rhs=SK[:, 0],
                                         start=False, stop=False)
                        nc.tensor.matmul(so, lhsT=blkEn[m], rhs=SK[:, 1],
                                         start=False, stop=True)
                        nc.scalar.activation(expST[32 * m:32 * m + 12], so, AF.Exp,
                                             accum_out=Zbuf[32 * m:32 * m + 12, t:t + 1])
                    for j in range(4):
                        etp = psum.tile([128, 44], bf16, tag="ps", bufs=3)
                        nc.tensor.transpose(etp, expST[:, j * 128:(j + 1) * 128],
                                            ident_bf[:44, :44])
                        eT = sb.tile([128, 44], bf16, tag="eT", bufs=4)
                        nc.vector.tensor_copy(eT, etp)
                        nc.tensor.matmul(afp, lhsT=eT, rhs=frow[:, 4 * t + j],
                                         start=(t == 0 and j == 0),
                                         stop=(t == NT - 1 and j == 3))

                # ---- per-module epilogue: normalize, project, LN ----
                for m in range(2):
                    lm = m * 2 + l
                    Zs = qs.tile([12, 1], f32, tag="Zs")
                    nc.vector.tensor_reduce(out=Zs, in_=Zbuf[32 * m:32 * m + 12, :],
                                            axis=AX.X, op=OP.add)
                    Zr = qs.tile([12, 1], f32, tag="Zr")
                    nc.vector.reciprocal(out=Zr, in_=Zs)
                    af = qs.tile([12, 2, 120], f32, tag="af_sb")
                    for dc in range(2):
                        nc.vector.tensor_scalar_mul(
                            af[:, dc], afp[32 * m:32 * m + 12, dc * 128:dc * 128 + 120], Zr)
                    afT = qs.tile([120, 2, 12], f32, tag="afT")
                    for dc in range(2):
                        atp = psum.tile([120, 12], f32, tag="ps", bufs=3)
                        nc.tensor.transpose(atp, af[:, dc],
                                            ident[:12, :12])
                        nc.vector.tensor_copy(afT[:, dc], atp)
                    avsb = qs.tile([120, 2, 12], f32, tag="avsb")
                    for dc in range(2):
                        avp = psum.tile([120, 12], f32, tag="ps", bufs=3)
                        for cc in range(2):
                            nc.tensor.matmul(
                                avp, lhsT=W["wv"][:, lm, cc, dc * 120:(dc + 1) * 120],
                                rhs=afT[:, cc], start=(cc == 0), stop=(cc == 1))
                        nc.vector.tensor_scalar_add(avsb[:, dc], avp,
                                                    W["bv"][:, lm, dc:dc + 1])
                    avsel = qs.tile([120, 2, NGP], f32, tag="avsel")
                    nc.sync.dma_start(out=avsel[0:80, 0], in_=avsb[0:80, 0, 0:4])
                    nc.sync.dma_start(out=avsel[80:120, 0], in_=avsb[80:120, 0, 4:8])
                    nc.sync.dma_start(out=avsel[0:40, 1], in_=avsb[0:40, 1, 4:8])
                    nc.sync.dma_start(out=avsel[40:120, 1], in_=avsb[40:120, 1, 8:12])
                    osp = psum.tile([NGP, 240], f32, tag="ps", bufs=3)
                    for dc in range(2):
                        nc.tensor.matmul(osp, lhsT=avsel[:, dc], rhs=W["wo"][:, lm, dc],
                                         start=(dc == 0), stop=(dc == 1))
                    gbt = qs.tile([NGP, 3, 240], f32, tag="gbt")
                    nc.gpsimd.dma_start(out=gbt, in_=bcast(w_h["gbb"][lm], NGP))
                    xn = qs.tile([NGP, 240], f32, tag=f"x_{m}")
                    nc.vector.tensor_tensor(out=xn, in0=osp, in1=xstate[m], op=OP.add)
                    nc.vector.tensor_tensor(out=xn, in0=xn, in1=gbt[:, 0], op=OP.add)
                    # LayerNorm
                    stats = qs.tile([NGP, 6], f32, tag="stats")
                    nc.vector.bn_stats(out=stats, in_=xn)
                    mv = qs.tile([NGP, 2], f32, tag="mv")
                    nc.vector.bn_aggr(out=mv, in_=stats)
                    lnv = qs.tile([NGP, 1], f32, tag="lnv")
                    nc.scalar.activation(lnv, mv[:, 1:2], AF.Ln, bias=epst)
                    rstd = qs.tile([NGP, 1], f32, tag="rstd")
                    nc.scalar.activation(rstd, lnv, AF.Exp, scale=-0.5)
                    nc.vector.tensor_scalar(out=xn, in0=xn, scalar1=mv[:, 0:1],
                                            scalar2=rstd, op0=OP.subtract, op1=OP.mult)
                    nc.vector.tensor_tensor(out=xn, in0=xn, in1=gbt[:, 1], op=OP.mult)
                    nc.vector.tensor_tensor(out=xn, in0=xn, in1=gbt[:, 2], op=OP.add)
                    xstate[m] = xn

            # ---- write outputs for batch b ----
            nc.sync.dma_start(
                out=out_h[b, 0:NGP * D].rearrange("(q d) -> q d", q=NGP),
                in_=xstate[0])
            nc.sync.dma_start(
                out=out_h[b, NGP * D + 64:OUTF].rearrange("(q d) -> q d", q=NGP),
                in_=xstate[1])

    nc.finalize()
    return nc


_CACHE = {}


def kernel(point_cloud, gripper_pcd, goal_gripper_pcd, agent_pos, params):
    from concourse.bass_utils import run_bass_kernel_spmd

    point_cloud = np.ascontiguousarray(point_cloud, F32)
    gripper_pcd = np.ascontiguousarray(gripper_pcd, F32)
    goal_gripper_pcd = np.ascontiguousarray(goal_gripper_pcd, F32)
    agent_pos = np.ascontiguousarray(agent_pos, F32)

    consts = _prep_consts(params)
    if "nc" not in _CACHE:
        _CACHE["nc"] = _build_program(consts)
    nc = _CACHE["nc"]

    in_maps = []
    for c in range(NC_COUNT):
        s = slice(c * BL, (c + 1) * BL)
        m = {"pc": point_cloud[s], "grip": gripper_pcd[s],
             "goal": goal_gripper_pcd[s], "agent": agent_pos[s]}
        for name, arr in consts.items():
            m[name] = arr
        in_maps.append(m)

    res = run_bass_kernel_spmd(nc, in_maps, list(range(NC_COUNT)),
                               trace=bool(_CACHE.get("trace", False)))
    _CACHE["last_result"] = res
    return np.concatenate([r["out"] for r in res.results], axis=0)


# revision 30
# speedup vs baseline: 1775.6854x; 1.1030x over previous
"""Trainium2 Bass kernel for nn_Act3dEncoder (dense_transformer).

Sharding: data-parallel over batch B=32 across 8 NeuronCores (4 batches/core),
params replicated. All compute on device. Key algorithmic restructurings
(validated against the reference to ~7e-7 in fp32):

 - Activations kept feature-transposed ([feat, n] on partitions) so every GEMM
   contracts on partitions with the weights stationary.
 - Feature dims de-interleaved (evens|odds) via host-side row permutation of
   Wq/Wk so RoPE pair structure becomes two aligned [120, n] chunks.
 - Rotation folded into the PSUM evacuation: CK/SK = (K + bk) * cos/sin via
   one scalar_tensor_tensor pass each; scores computed as a 4-chunk augmented
   contraction against head-blocked rotated queries (zeros off-head), which
   also realizes the per-head block-diagonal attention in one matmul.
 - attn@V collapsed via attnF: af = softmax-unnormalized(exp(s)) @ feat, then
   av = (af/Z) @ Wv^T + bv (softmax rows sum to 1). Eliminates all V
   projections over N entirely.
 - Scores are small (|s| < 0.5 for this model), so exp() without
   max-subtraction; Z accumulated for free via the ACT accum_out port.
 - fp32r (full-rate) for the big matmuls, bf16 for cos/sin tables, featrow and
   exp(scores); everything else fp32.
"""

import sys

sys.path.insert(0, "/opt/trn_rl_repo")

import numpy as np
import ml_dtypes

B, N, D, H, NGP = 32, 8192, 240, 3, 4
HD = D // H          # 80
NC_COUNT = 8
BL = B // NC_COUNT   # 4 batches per core
PAIRS = D // 2       # 120
T = 512              # n-tile
NT = N // T          # 16
OUTF = NGP * D + 64 + NGP * D  # 1984
F32 = np.float32

_PERM = np.concatenate([np.arange(0, D, 2), np.arange(1, D, 2)])  # evens|odds


def _prep_consts(params):
    """Host-side weight layout prep. Returns dict name -> np.ndarray."""
    c = {}
    vis = params["vision"]
    # vision layer 1: W [240, 3] -> lhsT [3, 240]
    c["w1t"] = np.ascontiguousarray(vis[0]["W"].T, F32)  # [3, 240]
    for i, nm in ((1, "w2t"), (2, "w3t")):
        wt = vis[i]["W"].T.astype(F32)  # [240 in, 240 out]
        c[nm] = np.ascontiguousarray(
            wt.reshape(2, 120, 240).transpose(1, 0, 2))  # [cp, cc, out]
    vb = np.stack([vis[i]["b"].astype(F32) for i in range(3)], -1)  # [240, 3]
    c["vb"] = np.ascontiguousarray(vb.reshape(2, 120, 3).transpose(1, 0, 2))  # [120,2,3]

    # RoPE frequency selection matrix: ang[p, n] = sum_a msel[a, p] * xyz[a, n]
    d3 = D // 3
    div = np.exp(np.arange(0, d3, 2, dtype=F32) * (-np.log(10000.0) / d3))  # [40]
    msel = np.zeros((3, PAIRS), F32)
    for p in range(PAIRS):
        msel[p // 40, p] = div[p % 40]
    c["msel"] = msel
    c["msel_r"] = msel.copy()

    # attention weights: lm = module*2 + layer
    wq = np.zeros((120, 4, 2, 240), F32)
    wk = np.zeros((121, 4, 2, 240), F32)
    wv = np.zeros((120, 4, 2, 240), F32)
    wo = np.zeros((120, 4, 2, 240), F32)
    bq = np.zeros((120, 4, 2), F32)
    bk = np.zeros((120, 4, 2), F32)
    bv = np.zeros((120, 4, 2), F32)
    gbb = np.zeros((4, 3, 240), F32)  # [lm, (bo,g,beta), d]
    scale = HD ** -0.5
    for m, key in ((0, "attn"), (1, "goal_attn")):
        for l, p in enumerate(params[key]):
            lm = m * 2 + l
            Wq, Wk, Wv = np.split(np.asarray(p["Wqkv"], F32), 3, axis=0)
            bq_, bk_, bv_ = np.split(np.asarray(p["bqkv"], F32), 3)
            wqt = (Wq * scale).T[:, _PERM]   # [240c, 240 out-permuted]
            wkt = Wk.T[:, _PERM]
            wvt = Wv.T                        # natural out
            wot = np.asarray(p["Wo"], F32).T  # [240 in(avsel d), 240 out]
            for arr, w in ((wq, wqt), (wv, wvt), (wo, wot)):
                arr[:, lm] = w.reshape(2, 120, 240).transpose(1, 0, 2)
            wk[0:120, lm] = wkt.reshape(2, 120, 240).transpose(1, 0, 2)
            wk[120, lm, 0] = bk_[_PERM]  # bias via constant-ones feature row
            bq[:, lm] = (bq_ * scale)[_PERM].reshape(2, 120).T
            bk[:, lm] = bk_[_PERM].reshape(2, 120).T
            bv[:, lm] = bv_.reshape(2, 120).T
            gbb[lm, 0] = p["bo"]
            gbb[lm, 1] = p["g"]
            gbb[lm, 2] = p["beta"]
    c["wq"], c["wv"], c["wo"] = wq, wv, wo
    c["wk"] = wk.astype(ml_dtypes.bfloat16)
    c["bq"], c["bk"], c["bv"], c["gbb"] = bq, bk, bv, gbb

    # gripper / goal PE MLPs [6 -> 128 -> 256 -> 80]
    gw1 = np.zeros((6, 2, 128), F32)
    gb1 = np.zeros((128, 2), F32)
    gw2 = np.zeros((128, 2, 256), F32)
    gb2 = np.zeros((128, 4), F32)   # [p, which*2 + oc]
    gw3 = np.zeros((128, 2, 2, 80), F32)
    gb3 = np.zeros((80, 2), F32)
    for w, key in ((0, "gripper_pe_mlp"), (1, "goal_pe_mlp")):
        ps = params[key]
        gw1[:, w] = np.asarray(ps[0]["W"], F32).T
        gb1[:, w] = ps[0]["b"]
        gw2[:, w] = np.asarray(ps[1]["W"], F32).T
        gb2[:, w * 2 + 0] = ps[1]["b"][:128]
        gb2[:, w * 2 + 1] = ps[1]["b"][128:]
        w3t = np.asarray(ps[2]["W"], F32).T  # [256, 80]
        gw3[:, w, 0] = w3t[:128]
        gw3[:, w, 1] = w3t[128:]
        gb3[:, w] = ps[2]["b"]
    c["gw1"], c["gb1"], c["gw2"], c["gb2"] = gw1, gb1, gw2, gb2
    c["gw3"], c["gb3"] = gw3, gb3
    c["embeds"] = np.stack([np.asarray(params["embed"], F32),
                            np.asarray(params["goal_embed"], F32)])  # [2, 160]

    st = params["state_mlp"]
    c["sw1"] = np.ascontiguousarray(np.asarray(st[0]["W"], F32).T)  # [8, 64]
    c["sw2"] = np.ascontiguousarray(np.asarray(st[1]["W"], F32).T)  # [64, 64]
    c["sb"] = np.stack([np.asarray(st[0]["b"], F32),
                        np.asarray(st[1]["b"], F32)], -1)  # [64, 2]
    return c


def _build_program(consts):
    import ml_dtypes
    import concourse.bass as bass
    import concourse.bacc as bacc
    import concourse.mybir as mybir
    import concourse.tile as tile
    from concourse.masks import make_identity
    from contextlib import ExitStack

    f32 = mybir.dt.float32
    f32r = mybir.dt.float32r
    bf16 = mybir.dt.bfloat16
    AF = mybir.ActivationFunctionType
    OP = mybir.AluOpType
    AX = mybir.AxisListType

    nc = bacc.Bacc(None, target_bir_lowering=False)

    pc_h = nc.declare_dram_parameter("pc", [BL, N, 3], f32, isOutput=False)
    grip_h = nc.declare_dram_parameter("grip", [BL, NGP, 3], f32, isOutput=False)
    goal_h = nc.declare_dram_parameter("goal", [BL, NGP, 3], f32, isOutput=False)
    agent_h = nc.declare_dram_parameter("agent", [BL, 8], f32, isOutput=False)
    F32R_W = {"w1t", "w2t", "w3t", "msel_r"}
    w_h = {}
    for name, arr in consts.items():
        if arr.dtype == ml_dtypes.bfloat16:
            dt_ = bf16
        elif name in F32R_W:
            dt_ = f32r
        else:
            dt_ = f32
        w_h[name] = nc.declare_dram_parameter(name, list(arr.shape), dt_,
                                              isOutput=False)
    out_h = nc.declare_dram_parameter("out", [BL, OUTF], f32, isOutput=True)

    def bcast(handle, p):
        ap = handle[:]
        return bass.AP(tensor=ap.tensor, offset=ap.offset,
                       ap=[[0, p]] + list(ap.ap))

    def r(ap):
        return ap.bitcast(f32r)

    with tile.TileContext(nc) as tc, ExitStack() as ctx:
        wp = ctx.enter_context(tc.tile_pool(name="wp", bufs=1))
        persist = ctx.enter_context(tc.tile_pool(name="persist", bufs=1))
        sb = ctx.enter_context(tc.tile_pool(name="sb", bufs=2))
        ck = ctx.enter_context(tc.tile_pool(name="ck", bufs=2))
        qs = ctx.enter_context(tc.tile_pool(name="qs", bufs=2))
        psum = ctx.enter_context(tc.tile_pool(name="psum", bufs=2, space="PSUM"))

        # ---- load weights to SBUF ----
        W = {}
        for name, arr in consts.items():
            if arr.dtype == ml_dtypes.bfloat16:
                dt_ = bf16
            elif name in F32R_W:
                dt_ = f32r
            else:
                dt_ = f32
            tile_ = wp.tile(list(arr.shape), dt_, tag=f"w_{name}", name=f"w_{name}")
            nc.sync.dma_start(out=tile_, in_=w_h[name][:])
            W[name] = tile_
        ident = wp.tile([128, 128], f32, tag="ident")
        make_identity(nc, ident)
        ident_bf = wp.tile([128, 128], bf16, tag="ident_bf")
        make_identity(nc, ident_bf)
        pihalf = wp.tile([128, 1], f32, tag="pihalf")
        nc.vector.memset(pihalf, float(np.pi / 2))
        epst = wp.tile([NGP, 1], f32, tag="epst")
        nc.vector.memset(epst, 1e-5)
        zblk = wp.tile([120, 12], bf16, tag="zblk")
        nc.vector.memset(zblk, 0.0)
        # broadcast constants over rows
        emb_s = wp.tile([NGP, 2, 160], f32, tag="emb_s")
        nc.gpsimd.dma_start(out=emb_s, in_=bcast(w_h["embeds"], NGP))

        # ---- state MLP (all 4 batches at once) ----
        sT = qs.tile([8, BL], f32, tag="sT")
        with nc.allow_non_contiguous_dma(reason="tiny transposed load"):
            nc.sync.dma_start(out=sT, in_=agent_h[:].rearrange("b f -> f b"))
        s1p = psum.tile([64, BL], f32, tag="ps", bufs=3)
        nc.tensor.matmul(s1p, lhsT=W["sw1"][:], rhs=sT, start=True, stop=True)
        s1 = qs.tile([64, BL], f32, tag="s1")
        nc.scalar.activation(s1, s1p, AF.Relu, bias=W["sb"][:, 0:1])
        s2p = psum.tile([64, BL], f32, tag="ps", bufs=3)
        nc.tensor.matmul(s2p, lhsT=W["sw2"][:], rhs=s1, start=True, stop=True)
        s2 = qs.tile([64, BL], f32, tag="s2")
        nc.vector.tensor_scalar_add(s2, s2p, W["sb"][:, 1:2])
        strp = psum.tile([BL, 64], f32, tag="ps", bufs=3)
        nc.tensor.transpose(strp, s2, ident[:64, :64])
        strow = qs.tile([BL, 64], f32, tag="strow")
        nc.vector.tensor_copy(strow, strp)
        nc.sync.dma_start(out=out_h[:, NGP * D:NGP * D + 64], in_=strow)

        featT = [persist.tile([128, N], bf16, tag=f"featT{cc}", name=f"featT{cc}")
                 for cc in range(2)]
        nc.vector.memset(featT[0][96:128, :], 1.0)  # row 120 = bias ones; 96:120 overwritten by L3

        for b in range(BL):
            # ======== vision + rope tables for batch b ========
            cosP = persist.tile([PAIRS, N], bf16, tag="cosP")
            sinP = persist.tile([PAIRS, N], bf16, tag="sinP")
            frow = persist.tile([128, N // 128, 256], bf16, tag="frow")

            pct = persist.tile([128, N // 128, 3], f32, tag="pct")
            nc.sync.dma_start(
                out=pct, in_=pc_h[b].rearrange("(cb p) c -> p cb c", p=128))

            for t in range(NT):
                n0 = t * T
                xyzT = sb.tile([3, T], f32r, tag="xyzT", bufs=3)
                for j in range(4):
                    cb = t * 4 + j
                    tp = psum.tile([3, 128], f32, tag="ps", bufs=3)
                    nc.tensor.transpose(tp, pct[:, cb, :], ident)
                    nc.vector.tensor_copy(xyzT[:, j * 128:(j + 1) * 128], tp)
                # rope angles -> cos/sin
                angp = psum.tile([PAIRS, T], f32, tag="af", bufs=1)
                nc.tensor.matmul(angp, lhsT=W["msel_r"][:], rhs=xyzT,
                                 start=True, stop=True)
                nc.scalar.activation(sinP[:, n0:n0 + T], angp, AF.Sin)
                nc.scalar.activation(cosP[:, n0:n0 + T], angp, AF.Sin,
                                     bias=pihalf[:PAIRS])
                # vision MLP (each layer: one 2-bank psum tile, 2 matmuls, 2 evacs)
                x1 = sb.tile([120, 2, T], f32r, tag="x1")
                p1 = [psum.tile([120, T], f32, tag="K", bufs=4, name=f"p1{o}") for o in range(2)]
                for oc in range(2):
                    nc.tensor.matmul(p1[oc], lhsT=W["w1t"][:, oc * 120:(oc + 1) * 120],
                                     rhs=xyzT, start=True, stop=True)
                for oc in range(2):
                    nc.vector.tensor_scalar(
                        out=x1[:, oc], in0=p1[oc], scalar1=W["vb"][:, oc, 0:1],
                        scalar2=0.0, op0=OP.add, op1=OP.max)
                x2 = sb.tile([120, 2, T], f32r, tag="x2")
                p2 = [psum.tile([120, T], f32, tag="K", bufs=4, name=f"p2{o}") for o in range(2)]
                for oc in range(2):
                    for cc in range(2):
                        nc.tensor.matmul(
                            p2[oc], lhsT=W["w2t"][:, cc, oc * 120:(oc + 1) * 120],
                            rhs=x1[:, cc], start=(cc == 0), stop=(cc == 1))
                for oc in range(2):
                    nc.scalar.activation(x2[:, oc], p2[oc], AF.Relu,
                                         bias=W["vb"][:, oc, 1:2])
                p3 = [psum.tile([120, T], f32, tag="K", bufs=4, name=f"p3{o}") for o in range(2)]
                for oc in range(2):
                    for cc in range(2):
                        nc.tensor.matmul(
                            p3[oc], lhsT=W["w3t"][:, cc, oc * 120:(oc + 1) * 120],
                            rhs=x2[:, cc], start=(cc == 0), stop=(cc == 1))
                nc.vector.tensor_scalar_add(featT[0][0:120, n0:n0 + T], p3[0],
                                            W["vb"][:, 0, 2:3])
                nc.scalar.activation(featT[1][0:120, n0:n0 + T], p3[1], AF.Identity,
                                     bias=W["vb"][:, 1, 2:3])
            # featrow via PE transposes (xbar DMA transpose hangs HW here)
            for cb in range(N // 128):
                for cc in range(2):
                    fp = psum.tile([128, 120], bf16, tag="ps", bufs=3, name="fp")
                    nc.tensor.transpose(fp, featT[cc][0:120, cb * 128:(cb + 1) * 128],
                                        ident_bf[:120, :120])
                    if (cb + cc) % 2 == 0:
                        nc.scalar.copy(frow[:, cb, cc * 128:cc * 128 + 120], fp)
                    else:
                        nc.vector.tensor_copy(frow[:, cb, cc * 128:cc * 128 + 120], fp)

            # ======== gripper-side module inputs ========
            in6 = {}
            cosq, sinq = {}, {}
            xstate = {}
            for m in range(2):
                in6[m] = qs.tile([6, NGP], f32, tag=f"in6_{m}", name=f"in6_{m}")
            gxyz = qs.tile([3, NGP], f32, tag="gxyz")
            oxyz = qs.tile([3, NGP], f32, tag="oxyz")
            dispt = qs.tile([3, NGP], f32, tag="dispt")
            with nc.allow_non_contiguous_dma(reason="tiny transposed load"):
                nc.sync.dma_start(out=gxyz, in_=grip_h[b].rearrange("q c -> c q"))
                nc.sync.dma_start(out=oxyz, in_=goal_h[b].rearrange("q c -> c q"))
            nc.vector.tensor_tensor(out=dispt, in0=oxyz, in1=gxyz, op=OP.subtract)
            nc.sync.dma_start(out=in6[0][0:3], in_=gxyz)
            nc.sync.dma_start(out=in6[0][3:6], in_=dispt)
            nc.sync.dma_start(out=in6[1][0:3], in_=oxyz)
            nc.sync.dma_start(out=in6[1][3:6], in_=dispt)

            for m in range(2):
                # query rope tables
                aqp = psum.tile([PAIRS, NGP], f32, tag="ps", bufs=3)
                nc.tensor.matmul(aqp, lhsT=W["msel"][:], rhs=in6[m][0:3],
                                 start=True, stop=True)
                sinq[m] = qs.tile([PAIRS, NGP], f32, tag=f"sinq{m}", name=f"sinq{m}")
                cosq[m] = qs.tile([PAIRS, NGP], f32, tag=f"cosq{m}", name=f"cosq{m}")
                nc.scalar.activation(sinq[m], aqp, AF.Sin)
                nc.scalar.activation(cosq[m], aqp, AF.Sin, bias=pihalf[:PAIRS])
                # gripper PE MLP
                g1p = psum.tile([128, NGP], f32, tag="ps", bufs=3)
                nc.tensor.matmul(g1p, lhsT=W["gw1"][:, m], rhs=in6[m],
                                 start=True, stop=True)
                g1 = qs.tile([128, NGP], f32, tag="g1")
                nc.scalar.activation(g1, g1p, AF.Relu, bias=W["gb1"][:, m:m + 1])
                g2 = qs.tile([128, 2, NGP], f32, tag="g2")
                for oc in range(2):
                    g2p = psum.tile([128, NGP], f32, tag="ps", bufs=3)
                    nc.tensor.matmul(g2p, lhsT=W["gw2"][:, m, oc * 128:(oc + 1) * 128],
                                     rhs=g1, start=True, stop=True)
                    nc.scalar.activation(g2[:, oc], g2p, AF.Relu,
                                         bias=W["gb2"][:, 2 * m + oc:2 * m + oc + 1])
                g3p = psum.tile([80, NGP], f32, tag="ps", bufs=3)
                for cc in range(2):
                    nc.tensor.matmul(g3p, lhsT=W["gw3"][:, m, cc], rhs=g2[:, cc],
                                     start=(cc == 0), stop=(cc == 1))
                gpeT = qs.tile([80, NGP], f32, tag="gpeT")
                nc.vector.tensor_scalar_add(gpeT, g3p, W["gb3"][:, m:m + 1])
                gperp = psum.tile([NGP, 80], f32, tag="ps", bufs=3)
                nc.tensor.transpose(gperp, gpeT, ident[:80, :80])
                gf = qs.tile([NGP, 240], f32, tag=f"x_{m}")
                nc.vector.tensor_copy(gf[:, 0:160], emb_s[:, m])
                nc.vector.tensor_copy(gf[:, 160:240], gperp)
                xstate[m] = gf

            # ======== attention layers ========
            for l in range(2):
                blkE, blkO, blkEn = {}, {}, {}
                for m in range(2):
                    lm = m * 2 + l
                    x = xstate[m]
                    xT = qs.tile([120, 2, NGP], f32, tag=f"xT{m}")
                    for cc in range(2):
                        xtp = psum.tile([120, NGP], f32, tag="ps", bufs=3)
                        nc.tensor.transpose(xtp, x[:, cc * 120:(cc + 1) * 120],
                                            ident[:NGP, :NGP])
                        nc.vector.tensor_copy(xT[:, cc], xtp)
                    qsb = qs.tile([120, 2, NGP], f32, tag=f"qsb{m}")
                    for oc in range(2):
                        qp = psum.tile([120, NGP], f32, tag="ps", bufs=3)
                        for cc in range(2):
                            nc.tensor.matmul(
                                qp, lhsT=W["wq"][:, lm, cc, oc * 120:(oc + 1) * 120],
                                rhs=xT[:, cc], start=(cc == 0), stop=(cc == 1))
                        nc.vector.tensor_scalar_add(qsb[:, oc], qp,
                                                    W["bq"][:, lm, oc:oc + 1])
                    t1 = qs.tile([120, NGP], f32, tag="t1")
                    t2 = qs.tile([120, NGP], f32, tag="t2")
                    qrE = qs.tile([120, NGP], bf16, tag=f"qrE{m}")
                    qrO = qs.tile([120, NGP], bf16, tag=f"qrO{m}")
                    nc.vector.tensor_tensor(out=t1, in0=qsb[:, 0], in1=cosq[m], op=OP.mult)
                    nc.vector.tensor_tensor(out=t2, in0=qsb[:, 1], in1=sinq[m], op=OP.mult)
                    nc.vector.tensor_tensor(out=qrE, in0=t1, in1=t2, op=OP.subtract)
                    nc.vector.tensor_tensor(out=t1, in0=qsb[:, 1], in1=cosq[m], op=OP.mult)
                    nc.vector.tensor_tensor(out=t2, in0=qsb[:, 0], in1=sinq[m], op=OP.mult)
                    nc.vector.tensor_tensor(out=qrO, in0=t1, in1=t2, op=OP.add)
                    bE = qs.tile([120, 12], bf16, tag=f"blkE{m}")
                    bO = qs.tile([120, 12], bf16, tag=f"blkO{m}")
                    bEn = qs.tile([120, 12], bf16, tag=f"blkEn{m}")
                    nc.sync.dma_start(out=bE, in_=zblk)
                    nc.sync.dma_start(out=bO, in_=zblk)
                    for h in range(3):
                        nc.sync.dma_start(
                            out=bE[40 * h:40 * (h + 1), 4 * h:4 * (h + 1)],
                            in_=qrE[40 * h:40 * (h + 1), :])
                        nc.sync.dma_start(
                            out=bO[40 * h:40 * (h + 1), 4 * h:4 * (h + 1)],
                            in_=qrO[40 * h:40 * (h + 1), :])
                    nc.vector.tensor_scalar_mul(bEn, bE, -1.0)
                    blkE[m], blkO[m], blkEn[m] = bE, bO, bEn

                Zbuf = qs.tile([44, NT], f32, tag="Zbuf", name="Zbuf")
                # merged A+G af accumulator: rows 0:12 = module A, 32:44 = G
                afp = psum.tile([44, 256], f32, tag="af", name="afp", bufs=1)

                for t in range(NT):
                    n0 = t * T
                    expST = sb.tile([44, T], bf16, tag="expST", bufs=4)
                    for m in range(2):
                        lm = m * 2 + l
                        kps = [psum.tile([120, T], f32, tag="K", name=f"kps{oc_}",
                                         bufs=4) for oc_ in range(2)]
                        for oc in range(2):
                            nc.tensor.matmul(
                                kps[oc],
                                lhsT=W["wk"][0:121, lm, 0, oc * 120:(oc + 1) * 120],
                                rhs=featT[0][0:121, n0:n0 + T],
                                start=True, stop=False)
                            nc.tensor.matmul(
                                kps[oc],
                                lhsT=W["wk"][0:120, lm, 1, oc * 120:(oc + 1) * 120],
                                rhs=featT[1][0:120, n0:n0 + T],
                                start=False, stop=True)
                        kb = sb.tile([120, 2, T], bf16, tag="kb", bufs=4)
                        nc.scalar.copy(kb[:, 0], kps[0])
                        nc.vector.tensor_copy(kb[:, 1], kps[1])
                        CK = ck.tile([120, 2, T], bf16, tag="CK", bufs=4)
                        SK = ck.tile([120, 2, T], bf16, tag="SK", bufs=4)
                        cp = cosP[:, n0:n0 + T]
                        sp_ = sinP[:, n0:n0 + T]
                        cpb = bass.AP(tensor=cp.tensor, offset=cp.offset,
                                      ap=[list(cp.ap[0]), [0, 2], list(cp.ap[1])])
                        spb = bass.AP(tensor=sp_.tensor, offset=sp_.offset,
                                      ap=[list(sp_.ap[0]), [0, 2], list(sp_.ap[1])])
                        nc.vector.tensor_tensor(out=CK, in0=kb, in1=cpb, op=OP.mult)
                        nc.vector.tensor_tensor(out=SK, in0=kb, in1=spb, op=OP.mult)
                        so = psum.tile([12, T], f32, tag="ps", bufs=3, name="so")
                        nc.tensor.matmul(so, lhsT=blkE[m], rhs=CK[:, 0],
                                         start=True, stop=False)
                        nc.tensor.matmul(so, lhsT=blkO[m], rhs=CK[:, 1],
                                         start=False, stop=False)
                        nc.tensor.matmul(so, lhsT=blkO[m], rhs=SK[:, 0],
                                         start=False, stop=False)
                        nc.tensor.matmul(so, lhsT=blkEn[m], rhs=SK[:, 1],
                                         start=False, stop=True)
                        nc.scalar.activation(expST[32 * m:32 * m + 12], so, AF.Exp,
                                             accum_out=Zbuf[32 * m:32 * m + 12, t:t + 1])
                    for j in range(4):
                        etp = psum.tile([128, 44], bf16, tag="ps", bufs=3)
                        nc.tensor.transpose(etp, expST[:, j * 128:(j + 1) * 128],
                                            ident_bf[:44, :44])
                        eT = sb.tile([128, 44], bf16, tag="eT", bufs=6)
                        nc.vector.tensor_copy(eT, etp)
                        nc.tensor.matmul(afp, lhsT=eT, rhs=frow[:, 4 * t + j],
                                         start=(t == 0 and j == 0),
                                         stop=(t == NT - 1 and j == 3))

                # ---- per-module epilogue: normalize, project, LN ----
                for m in range(2):
                    lm = m * 2 + l
                    Zs = qs.tile([12, 1], f32, tag="Zs")
                    nc.vector.tensor_reduce(out=Zs, in_=Zbuf[32 * m:32 * m + 12, :],
                                            axis=AX.X, op=OP.add)
                    Zr = qs.tile([12, 1], f32, tag="Zr")
                    nc.vector.reciprocal(out=Zr, in_=Zs)
                    af = qs.tile([12, 2, 120], f32, tag="af_sb")
                    for dc in range(2):
                        nc.vector.tensor_scalar_mul(
                            af[:, dc], afp[32 * m:32 * m + 12, dc * 128:dc * 128 + 120], Zr)
                    afT = qs.tile([120, 2, 12], f32, tag="afT")
                    for dc in range(2):
                        atp = psum.tile([120, 12], f32, tag="ps", bufs=3)
                        nc.tensor.transpose(atp, af[:, dc],
                                            ident[:12, :12])
                        nc.vector.tensor_copy(afT[:, dc], atp)
                    avsb = qs.tile([120, 2, 12], f32, tag="avsb")
                    for dc in range(2):
                        avp = psum.tile([120, 12], f32, tag="ps", bufs=3)
                        for cc in range(2):
                            nc.tensor.matmul(
                                avp, lhsT=W["wv"][:, lm, cc, dc * 120:(dc + 1) * 120],
                                rhs=afT[:, cc], start=(cc == 0), stop=(cc == 1))
                        nc.vector.tensor_scalar_add(avsb[:, dc], avp,
                                                    W["bv"][:, lm, dc:dc + 1])
                    avsel = qs.tile([120, 2, NGP], f32, tag="avsel")
                    nc.sync.dma_start(out=avsel[0:80, 0], in_=avsb[0:80, 0, 0:4])
                    nc.sync.dma_start(out=avsel[80:120, 0], in_=avsb[80:120, 0, 4:8])
                    nc.sync.dma_start(out=avsel[0:40, 1], in_=avsb[0:40, 1, 4:8])
                    nc.sync.dma_start(out=avsel[40:120, 1], in_=avsb[40:120, 1, 8:12])
                    osp = psum.tile([NGP, 240], f32, tag="ps", bufs=3)
                    for dc in range(2):
                        nc.tensor.matmul(osp, lhsT=avsel[:, dc], rhs=W["wo"][:, lm, dc],
                                         start=(dc == 0), stop=(dc == 1))
                    gbt = qs.tile([NGP, 3, 240], f32, tag="gbt")
                    nc.gpsimd.dma_start(out=gbt, in_=bcast(w_h["gbb"][lm], NGP))
                    xn = qs.tile([NGP, 240], f32, tag=f"x_{m}")
                    nc.vector.tensor_tensor(out=xn, in0=osp, in1=xstate[m], op=OP.add)
                    nc.vector.tensor_tensor(out=xn, in0=xn, in1=gbt[:, 0], op=OP.add)
                    # LayerNorm
                    stats = qs.tile([NGP, 6], f32, tag="stats")
                    nc.vector.bn_stats(out=stats, in_=xn)
                    mv = qs.tile([NGP, 2], f32, tag="mv")
                    nc.vector.bn_aggr(out=mv, in_=stats)
                    lnv = qs.tile([NGP, 1], f32, tag="lnv")
                    nc.scalar.activation(lnv, mv[:, 1:2], AF.Ln, bias=epst)
                    rstd = qs.tile([NGP, 1], f32, tag="rstd")
                    nc.scalar.activation(rstd, lnv, AF.Exp, scale=-0.5)
                    nc.vector.tensor_scalar(out=xn, in0=xn, scalar1=mv[:, 0:1],
                                            scalar2=rstd, op0=OP.subtract, op1=OP.mult)
                    nc.vector.tensor_tensor(out=xn, in0=xn, in1=gbt[:, 1], op=OP.mult)
                    nc.vector.tensor_tensor(out=xn, in0=xn, in1=gbt[:, 2], op=OP.add)
                    xstate[m] = xn

            # ---- write outputs for batch b ----
            nc.sync.dma_start(
                out=out_h[b, 0:NGP * D].rearrange("(q d) -> q d", q=NGP),
                in_=xstate[0])
            nc.sync.dma_start(
                out=out_h[b, NGP * D + 64:OUTF].rearrange("(q d) -> q d", q=NGP),
                in_=xstate[1])

    nc.finalize()
    return nc


_CACHE = {}


def kernel(point_cloud, gripper_pcd, goal_gripper_pcd, agent_pos, params):
    from concourse.bass_utils import run_bass_kernel_spmd

    point_cloud = np.ascontiguousarray(point_cloud, F32)
    gripper_pcd = np.ascontiguousarray(gripper_pcd, F32)
    goal_gripper_pcd = np.ascontiguousarray(goal_gripper_pcd, F32)
    agent_pos = np.ascontiguousarray(agent_pos, F32)

    consts = _prep_consts(params)
    if "nc" not in _CACHE:
        _CACHE["nc"] = _build_program(consts)
    nc = _CACHE["nc"]

    in_maps = []
    for c in range(NC_COUNT):
        s = slice(c * BL, (c + 1) * BL)
        m = {"pc": point_cloud[s], "grip": gripper_pcd[s],
             "goal": goal_gripper_pcd[s], "agent": agent_pos[s]}
        for name, arr in consts.items():
            m[name] = arr
        in_maps.append(m)

    res = run_bass_kernel_spmd(nc, in_maps, list(range(NC_COUNT)),
                               trace=bool(_CACHE.get("trace", False)))
    _CACHE["last_result"] = res
    return np.concatenate([r["out"] for r in res.results], axis=0)
